# revision 1
# baseline (speedup 1.0000x reference)
"""BiLSTM-CRF loss kernel for Trainium2 (8 NeuronCores, Bass/Tile).

Strategy
--------
Cores 0-3 run the FORWARD LSTM direction, cores 4-7 the BACKWARD direction
(fed time-reversed x), each over 16 of the 64 sequences (data-parallel over
batch within each direction).  Pair (c, c+4) handles the same 16 sequences.

Per core:
  A) GX = x @ W_ih^T + (b_ih + b_hh)  -- big bf16 matmul, output to DRAM
  B) sequential LSTM recurrence, weight-stationary bf16 matmuls
     (gates kept transposed: [4*HD rows, batch cols]); h stored bf16 in SBUF
  C) em partial = h @ W_tag_half^T (+ b_tag on fwd cores only); the two
     halves of each pair are summed with a 2-slot AllReduce (role masks make
     the SPMD program uniform), the bwd slot is read back time-reversed
  D) CRF: gold-path score via on-device one-hot masks + matmul reductions;
     partition function via probability-domain scan alpha_t = eem_t * E^T a
     with periodic rescaling; final AllReduce of the per-core loss partials.

Gate blocks are host-permuted from torch order (i,f,g,o) to (i,f,o,g) so the
three sigmoids run as one contiguous activation and tanh as another.
"""

import sys

sys.path.insert(0, "/opt/trn_rl_repo")

import numpy as np
import ml_dtypes
from contextlib import ExitStack

import concourse.bass as bass
import concourse.bacc as bacc
import concourse.tile as tile
import concourse.mybir as mybir

F32 = mybir.dt.float32
BF16 = mybir.dt.bfloat16
I32 = mybir.dt.int32
AFT = mybir.ActivationFunctionType
ALU = mybir.AluOpType
AXL = mybir.AxisListType

NCORES = 8
NPAIR = 4  # fwd cores 0..3, bwd cores 4..7


# ---------------------------------------------------------------------------
# program builder (SPMD: one program, per-core divergence is data only)
# ---------------------------------------------------------------------------

def build_program(b, S, E, HD, T, B_full, R=6, CH=16, NSL=512, reps=1, stop_after=None):
    """b: sequences per core; returns the Bass program."""
    KE = E // 128          # input-proj K tiles
    NH = HD // 128         # hidden K tiles (= h tiles)
    NM = 4 * NH            # gate m-tiles (permuted order i,f,o,g)
    SB = S * b             # (t, b) flattened column count
    NSL = min(NSL, SB)
    assert SB % NSL == 0 and S % CH == 0
    TSL = NSL // b         # timesteps per em/gx column slice
    W = NH * b             # per-gate column width in transposed gate layout

    nc = bacc.Bacc("TRN2", target_bir_lowering=False, debug=False,
                   num_devices=NCORES)

    # ---- I/O ----
    xT = nc.dram_tensor("xT", [KE, 128, SB], BF16, kind="ExternalInput")
    wihT = nc.dram_tensor("wihT", [KE, 128, 4 * HD], BF16, kind="ExternalInput")
    whhT = nc.dram_tensor("whhT", [NH, 128, 4 * HD], BF16, kind="ExternalInput")
    bias4 = nc.dram_tensor("bias4", [128, NM], F32, kind="ExternalInput")
    wtagT = nc.dram_tensor("wtagT", [NH, 128, T], BF16, kind="ExternalInput")
    tagb = nc.dram_tensor("tagb", [T, 1], F32, kind="ExternalInput")
    m0 = nc.dram_tensor("m0", [T, 1], F32, kind="ExternalInput")
    m1 = nc.dram_tensor("m1", [T, 1], F32, kind="ExternalInput")
    labT = nc.dram_tensor("labT", [S, b], I32, kind="ExternalInput")
    transm = nc.dram_tensor("transm", [T, T], F32, kind="ExternalInput")
    startv = nc.dram_tensor("startv", [T, 1], F32, kind="ExternalInput")
    endv = nc.dram_tensor("endv", [T, 1], F32, kind="ExternalInput")
    loss = nc.dram_tensor("loss", [1, 1], F32, kind="ExternalOutput")

    with tile.TileContext(nc) as tc, ExitStack() as top:
        dram = top.enter_context(tc.tile_pool(name="dram", bufs=1, space="DRAM"))
        gxd = dram.tile([NM, 128, SB], F32)
        hsd = dram.tile([128, S * W], BF16)
        emdb = dram.tile([2, T, SB], F32)
        emdbo = dram.tile([2, T, SB], F32)
        lossdb = dram.tile([1, 1], F32)
        lossout = dram.tile([1, 1], F32)

        crf = top.enter_context(tc.tile_pool(name="crf", bufs=1))
        for _rep in range(reps):
            em_full = crf.tile([T, SB], F32, tag="emfull")
            eem = crf.tile([T, SB], F32, tag="eem")

            abc = ExitStack()
            persist = abc.enter_context(tc.tile_pool(name="persist", bufs=1))
            whh_sb = persist.tile([128, NH * 4 * HD], BF16)
            bias_sb = persist.tile([128, NM], F32)
            nc.sync.dma_start(whh_sb[:], whhT[:])
            nc.sync.dma_start(bias_sb[:], bias4[:])

            # ---------------- Phase A: GX = x @ W_ih^T + bias ----------------
            with ExitStack() as ph:
                wp = ph.enter_context(tc.tile_pool(name="wih", bufs=1))
                xp = ph.enter_context(tc.tile_pool(name="xt", bufs=2))
                gp = ph.enter_context(tc.tile_pool(name="gxout", bufs=3))
                pp = ph.enter_context(tc.tile_pool(name="gxps", bufs=2, space="PSUM"))
                wih_sb = wp.tile([128, KE * 4 * HD], BF16)
                nc.sync.dma_start(wih_sb[:], wihT[:])
                for n in range(SB // NSL):
                    xt_sb = xp.tile([128, KE * NSL], BF16)
                    nc.sync.dma_start(xt_sb[:], xT[:, :, n * NSL:(n + 1) * NSL])
                    for mm in range(NM):
                        ps = pp.tile([128, NSL], F32)
                        for ke in range(KE):
                            nc.tensor.matmul(
                                ps[:],
                                wih_sb[:, ke * 4 * HD + mm * 128:
                                       ke * 4 * HD + (mm + 1) * 128],
                                xt_sb[:, ke * NSL:(ke + 1) * NSL],
                                start=(ke == 0), stop=(ke == KE - 1))
                        gxo = gp.tile([128, NSL], F32)
                        nc.vector.tensor_scalar_add(gxo[:], ps[:],
                                                    bias_sb[:, mm:mm + 1])
                        nc.sync.dma_start(gxd[mm, :, n * NSL:(n + 1) * NSL], gxo[:])

            if stop_after == 'A':
                with tc.tile_pool(name="bail", bufs=1) as bp:
                    bt = bp.tile([1, 1], F32)
                    nc.vector.tensor_copy(bt[:], bias_sb[0:1, 0:1])
                    nc.sync.dma_start(loss[:], bt[:])
                abc.close()
                continue

            # ---------------- Phase B: LSTM recurrence ----------------
            with ExitStack() as ph:
                gxp = ph.enter_context(tc.tile_pool(name="gxin", bufs=3))
                hp = ph.enter_context(tc.tile_pool(name="hchunk", bufs=2))
                tp = ph.enter_context(tc.tile_pool(name="steptmp", bufs=3))
                cp = ph.enter_context(tc.tile_pool(name="cstate", bufs=1))
                rp = ph.enter_context(tc.tile_pool(name="recps", bufs=2, space="PSUM"))
                c_sb = cp.tile([128, W], F32)
                gxch = None
                hch = None
                h_prev = None
                for t in range(S):
                    if t % CH == 0:
                        gxch = gxp.tile([128, NM * CH * b], F32)
                        nc.sync.dma_start(
                            gxch[:], gxd[:, :, t * b:(t + CH) * b])
                        hch = hp.tile([128, CH * W], BF16)
                    toff = (t % CH) * b

                    def gx_ap(mm_lo, mm_n):
                        return gxch[:].rearrange(
                            "p (m c) -> p m c", m=NM)[:, mm_lo:mm_lo + mm_n,
                                                      toff:toff + b]

                    if t == 0:
                        sig = tp.tile([128, 3 * W], F32, tag="sig")
                        nc.scalar.activation(sig[:].rearrange(
                            "p (m c) -> p m c", m=3 * NH), gx_ap(0, 3 * NH),
                            AFT.Sigmoid)
                        tg = tp.tile([128, W], F32, tag="tg")
                        nc.scalar.activation(tg[:].rearrange(
                            "p (m c) -> p m c", m=NH), gx_ap(3 * NH, NH), AFT.Tanh)
                        nc.vector.tensor_mul(c_sb[:], sig[:, 0:W], tg[:])
                    else:
                        ps = rp.tile([128, NM * b], F32)
                        for mm in range(NM):
                            for kt in range(NH):
                                nc.tensor.matmul(
                                    ps[:, mm * b:(mm + 1) * b],
                                    whh_sb[:, kt * 4 * HD + mm * 128:
                                           kt * 4 * HD + (mm + 1) * 128],
                                    h_prev[:, kt * b:(kt + 1) * b],
                                    start=(kt == 0), stop=(kt == NH - 1))
                        g_all = tp.tile([128, NM * b], F32, tag="gall")
                        nc.vector.tensor_tensor(
                            g_all[:].rearrange("p (m c) -> p m c", m=NM),
                            ps[:].rearrange("p (m c) -> p m c", m=NM),
                            gx_ap(0, NM), op=ALU.add)
                        sig = tp.tile([128, 3 * W], F32, tag="sig")
                        nc.scalar.activation(sig[:], g_all[:, 0:3 * W], AFT.Sigmoid)
                        tg = tp.tile([128, W], F32, tag="tg")
                        nc.scalar.activation(tg[:], g_all[:, 3 * W:], AFT.Tanh)
                        t1 = tp.tile([128, W], F32, tag="t1")
                        nc.vector.tensor_mul(t1[:], sig[:, W:2 * W], c_sb[:])
                        t2 = tp.tile([128, W], F32, tag="t2")
                        nc.vector.tensor_mul(t2[:], sig[:, 0:W], tg[:])
                        nc.vector.tensor_add(c_sb[:], t1[:], t2[:])
                    tanc = tp.tile([128, W], F32, tag="tanc")
                    nc.scalar.activation(tanc[:], c_sb[:], AFT.Tanh)
                    h_prev = hch[:, toff * NH:(toff + b) * NH]
                    nc.vector.tensor_mul(h_prev, sig[:, 2 * W:3 * W], tanc[:])
                    if t % CH == CH - 1:
                        nc.sync.dma_start(hsd[:, (t - CH + 1) * W:(t + 1) * W],
                                          hch[:])

            if stop_after == 'B':
                with tc.tile_pool(name="bail", bufs=1) as bp:
                    bt = bp.tile([1, 1], F32)
                    nc.vector.tensor_copy(bt[:], c_sb[0:1, 0:1])
                    nc.sync.dma_start(loss[:], bt[:])
                abc.close()
                continue

            # ---------------- Phase C: em partial + pair exchange ----------------
            with ExitStack() as ph:
                wp = ph.enter_context(tc.tile_pool(name="wtag", bufs=1))
                hsp = ph.enter_context(tc.tile_pool(name="hstream", bufs=2))
                ep = ph.enter_context(tc.tile_pool(name="emps", bufs=2, space="PSUM"))
                sp = ph.enter_context(tc.tile_pool(name="emtmp", bufs=2))
                wtag_sb = wp.tile([128, NH * T], BF16)
                nc.sync.dma_start(wtag_sb[:], wtagT[:])
                tagb_sb = wp.tile([T, 1], F32)
                nc.sync.dma_start(tagb_sb[:], tagb[:])
                m0_sb = wp.tile([T, 1], F32)
                nc.sync.dma_start(m0_sb[:], m0[:])
                m1_sb = wp.tile([T, 1], F32)
                nc.sync.dma_start(m1_sb[:], m1[:])
                for n in range(SB // NSL):
                    hse = hsp.tile([128, TSL * W], BF16)
                    nc.sync.dma_start(hse[:], hsd[:, n * TSL * W:(n + 1) * TSL * W])
                    hsv = hse[:].rearrange("p (t k c) -> p t k c", t=TSL, k=NH)
                    ps = ep.tile([T, NSL], F32)
                    for kt in range(NH):
                        nc.tensor.matmul(
                            ps[:].rearrange("p (t c) -> p t c", t=TSL),
                            wtag_sb[:, kt * T:(kt + 1) * T],
                            hsv[:, :, kt, :],
                            start=(kt == 0), stop=(kt == NH - 1))
                    s0 = sp.tile([T, NSL], F32, tag="s0")
                    nc.vector.tensor_scalar(s0[:], ps[:], tagb_sb[:], m0_sb[:],
                                            op0=ALU.add, op1=ALU.mult)
                    nc.sync.dma_start(emdb[0, :, n * NSL:(n + 1) * NSL], s0[:])
                    s1 = sp.tile([T, NSL], F32, tag="s1")
                    nc.vector.tensor_scalar(s1[:], ps[:], tagb_sb[:], m1_sb[:],
                                            op0=ALU.add, op1=ALU.mult)
                    nc.sync.dma_start(emdb[1, :, n * NSL:(n + 1) * NSL], s1[:])
                nc.gpsimd.collective_compute(
                    "AllReduce", ALU.add,
                    replica_groups=[[c, c + NPAIR] for c in range(NPAIR)],
                    ins=[emdb.opt()], outs=[emdbo.opt()])
            abc.close()

            nc.sync.dma_start(em_full[:], emdbo[0])
            nc.sync.dma_start(
                eem[:], emdbo[1].rearrange("j (t c) -> j t c", t=S)[:, ::-1, :])
            nc.vector.tensor_add(em_full[:], em_full[:], eem[:])

            if stop_after == 'C':
                with tc.tile_pool(name="bail", bufs=1) as bp:
                    bt = bp.tile([1, 1], F32)
                    nc.vector.tensor_copy(bt[:], em_full[0:1, 0:1])
                    nc.sync.dma_start(loss[:], bt[:])
                continue

            # ---------------- Phase D: CRF ----------------
            with ExitStack() as ph:
                sp = ph.enter_context(tc.tile_pool(name="crftmp", bufs=2))
                big = ph.enter_context(tc.tile_pool(name="crfbig", bufs=2))
                ap_ = ph.enter_context(tc.tile_pool(name="alphas", bufs=2))

                cst = sp.tile([T, T], F32, tag="cst")        # transitions
                nc.sync.dma_start(cst[:], transm[:])
                st_sb = sp.tile([T, 1], F32, tag="stv")
                nc.sync.dma_start(st_sb[:], startv[:])
                en_sb = sp.tile([T, 1], F32, tag="env")
                nc.sync.dma_start(en_sb[:], endv[:])

                # --- numerator ---
                lab9 = big.tile([T, SB], I32, tag="big")
                nc.sync.dma_start(
                    lab9[:],
                    labT[:].rearrange("s c -> (s c)")[None, :].broadcast_to((T, SB)))
                labf = big.tile([T, SB], F32, tag="big")
                nc.vector.tensor_copy(labf[:], lab9[:])
                io9 = sp.tile([T, 1], I32, tag="io9")
                nc.gpsimd.iota(io9[:], pattern=[[0, 1]], base=0, channel_multiplier=1)
                io9f = sp.tile([T, 1], F32, tag="io9f")
                nc.vector.tensor_copy(io9f[:], io9[:])
                onehot = big.tile([T, SB], F32, tag="big")
                nc.vector.tensor_scalar(onehot[:], labf[:], io9f[:], None,
                                        op0=ALU.is_equal)
                gmul = big.tile([T, SB], F32, tag="big")
                nc.vector.tensor_mul(gmul[:], onehot[:], em_full[:])
                acc = sp.tile([T, b], F32, tag="acc")
                nc.vector.tensor_reduce(
                    acc[:], gmul[:].rearrange("j (t c) -> j c t", c=b),
                    op=ALU.add, axis=AXL.X)
                # start/end gold scores
                stsc = sp.tile([T, b], F32, tag="stsc")
                nc.vector.tensor_scalar_mul(stsc[:], onehot[:, 0:b], st_sb[:])
                nc.vector.tensor_add(acc[:], acc[:], stsc[:])
                ensc = sp.tile([T, b], F32, tag="ensc")
                nc.vector.tensor_scalar_mul(ensc[:], onehot[:, (S - 1) * b:S * b],
                                            en_sb[:])
                nc.vector.tensor_add(acc[:], acc[:], ensc[:])
                # transition gold scores: TH = T^T @ onehot ; V = TH * onehot_next
                numps = ExitStack()
                pp = numps.enter_context(
                    tc.tile_pool(name="numps", bufs=2, space="PSUM"))
                for tc0 in range(0, S - 1, 32):
                    tn = min(32, S - 1 - tc0)
                    thp = pp.tile([T, 32 * b], F32, tag="thp")
                    nc.tensor.matmul(thp[:, 0:tn * b], cst[:],
                                     onehot[:, tc0 * b:(tc0 + tn) * b],
                                     start=True, stop=True)
                    v = sp.tile([T, 32 * b], F32, tag="v")
                    nc.vector.tensor_mul(v[:, 0:tn * b], thp[:, 0:tn * b],
                                         onehot[:, (tc0 + 1) * b:(tc0 + 1 + tn) * b])
                    vr = sp.tile([T, b], F32, tag="vr")
                    nc.vector.tensor_reduce(
                        vr[:], v[:, 0:tn * b].rearrange("j (t c) -> j c t", c=b),
                        op=ALU.add, axis=AXL.X)
                    nc.vector.tensor_add(acc[:], acc[:], vr[:])
                ones9 = sp.tile([T, 1], F32, tag="ones9")
                nc.vector.memset(ones9[:], 1.0)
                ones19 = sp.tile([1, T], F32, tag="ones19")
                nc.vector.memset(ones19[:], 1.0)
                nump = pp.tile([1, b], F32, tag="nump")
                nc.tensor.matmul(nump[:], ones9[:], acc[:], start=True, stop=True)
                num_sb = sp.tile([1, b], F32, tag="num")
                nc.vector.tensor_copy(num_sb[:], nump[:])
                numps.close()
                pp = ph.enter_context(tc.tile_pool(name="scanps", bufs=2, space="PSUM"))

                # --- partition function (probability-domain scan) ---
                Em = sp.tile([T, T], F32, tag="Em")
                nc.scalar.activation(Em[:], cst[:], AFT.Exp)
                nc.scalar.activation(eem[:], em_full[:], AFT.Exp)
                es = sp.tile([T, 1], F32, tag="es")
                nc.scalar.activation(es[:], st_sb[:], AFT.Exp)
                ee = sp.tile([T, 1], F32, tag="ee")
                nc.scalar.activation(ee[:], en_sb[:], AFT.Exp)
                logacc = sp.tile([1, b], F32, tag="logacc")
                nc.vector.memset(logacc[:], 0.0)
                alpha = ap_.tile([T, b], F32)
                nc.vector.tensor_scalar_mul(alpha[:], eem[:, 0:b], es[:])
                for t in range(1, S):
                    aps = pp.tile([T, b], F32, tag="aps")
                    nc.tensor.matmul(aps[:], Em[:], alpha[:], start=True, stop=True)
                    alpha = ap_.tile([T, b], F32)
                    nc.vector.tensor_mul(alpha[:], aps[:], eem[:, t * b:(t + 1) * b])
                    if t % R == 0 or t == S - 1:
                        ssum = pp.tile([1, b], F32, tag="ssum")
                        nc.tensor.matmul(ssum[:], ones9[:], alpha[:],
                                         start=True, stop=True)
                        ls = sp.tile([1, b], F32, tag="ls")
                        nc.scalar.activation(ls[:], ssum[:], AFT.Ln)
                        nc.vector.tensor_add(logacc[:], logacc[:], ls[:])
                        rc = sp.tile([1, b], F32, tag="rc")
                        nc.vector.reciprocal(rc[:], ssum[:])
                        bc = pp.tile([T, b], F32, tag="bc")
                        nc.tensor.matmul(bc[:], ones19[:], rc[:],
                                         start=True, stop=True)
                        a2 = ap_.tile([T, b], F32)
                        nc.vector.tensor_mul(a2[:], alpha[:], bc[:])
                        alpha = a2
                zp = pp.tile([1, b], F32, tag="aps")
                nc.tensor.matmul(zp[:], ee[:], alpha[:], start=True, stop=True)
                lz = sp.tile([1, b], F32, tag="lz")
                nc.scalar.activation(lz[:], zp[:], AFT.Ln)
                logz = sp.tile([1, b], F32, tag="logz")
                nc.vector.tensor_add(logz[:], lz[:], logacc[:])
                lv = sp.tile([1, b], F32, tag="lv")
                nc.vector.tensor_sub(lv[:], num_sb[:], logz[:])
                tot = sp.tile([1, 1], F32, tag="tot")
                nc.vector.tensor_reduce(tot[:], lv[:], op=ALU.add, axis=AXL.X)
                sc = sp.tile([1, 1], F32, tag="sc")
                nc.vector.tensor_scalar_mul(sc[:], tot[:], -1.0 / (2.0 * B_full))
                nc.sync.dma_start(lossdb[:], sc[:])
                nc.gpsimd.collective_compute(
                    "AllReduce", ALU.add,
                    replica_groups=[list(range(NCORES))],
                    ins=[lossdb.opt()], outs=[lossout.opt()])
                lf = sp.tile([1, 1], F32, tag="lf")
                nc.sync.dma_start(lf[:], lossout[:])
                nc.sync.dma_start(loss[:], lf[:])

    nc.compile()
    return nc


# ---------------------------------------------------------------------------
# host-side sharding
# ---------------------------------------------------------------------------

def _perm_ifog(HD):
    return np.concatenate([
        np.arange(0, HD), np.arange(HD, 2 * HD),
        np.arange(3 * HD, 4 * HD), np.arange(2 * HD, 3 * HD)])


def shard_inputs(inputs, b, S, E, HD, T):
    KE, NH = E // 128, HD // 128
    perm = _perm_ifog(HD)
    bf = ml_dtypes.bfloat16
    x = np.asarray(inputs["x"], np.float32)
    labels = np.asarray(inputs["labels"]).astype(np.int32)
    trans = np.asarray(inputs["transitions"], np.float32)
    startv = np.asarray(inputs["start_trans"], np.float32).reshape(T, 1)
    endv = np.asarray(inputs["end_trans"], np.float32).reshape(T, 1)
    Wtag = np.asarray(inputs["W_tag"], np.float32)
    btag = np.asarray(inputs["b_tag"], np.float32).reshape(T, 1)

    per_dir = {}
    for d, sfx in enumerate(("f", "b")):
        Wih = np.asarray(inputs[f"W_ih_{sfx}"], np.float32)[perm]
        Whh = np.asarray(inputs[f"W_hh_{sfx}"], np.float32)[perm]
        bias = (np.asarray(inputs[f"b_ih_{sfx}"], np.float32)
                + np.asarray(inputs[f"b_hh_{sfx}"], np.float32))[perm]
        per_dir[d] = dict(
            wihT=np.ascontiguousarray(
                Wih.T.reshape(KE, 128, 4 * HD)).astype(bf),
            whhT=np.ascontiguousarray(
                Whh.T.reshape(NH, 128, 4 * HD)).astype(bf),
            bias4=np.ascontiguousarray(
                bias.reshape(4 * NH, 128).T).astype(np.float32),
            wtagT=np.ascontiguousarray(
                Wtag[:, d * HD:(d + 1) * HD].T.reshape(NH, 128, T)).astype(bf),
            tagb=btag if d == 0 else np.zeros_like(btag),
            m0=np.full((T, 1), 1.0 - d, np.float32),
            m1=np.full((T, 1), float(d), np.float32),
        )

    in_maps = []
    for c in range(NCORES):
        d = c // NPAIR                      # 0 fwd, 1 bwd
        g = c % NPAIR                       # batch group
        xs = x[g * b:(g + 1) * b]           # (b, S, E)
        if d == 1:
            xs = xs[:, ::-1, :]
        xTc = np.ascontiguousarray(xs.transpose(2, 1, 0).reshape(KE, 128, S * b)
                                   ).astype(bf)
        m = dict(per_dir[d])
        m["xT"] = xTc
        m["labT"] = np.ascontiguousarray(labels[g * b:(g + 1) * b].T)
        m["transm"] = trans
        m["startv"] = startv
        m["endv"] = endv
        in_maps.append(m)
    return in_maps


# ---------------------------------------------------------------------------
# entry point
# ---------------------------------------------------------------------------

_B, _S, _E, _HD, _T = 64, 512, 1024, 512, 9
_cache = {}


def _get_program():
    if "nc" not in _cache:
        _cache["nc"] = build_program(_B // NPAIR, _S, _E, _HD, _T, _B)
    return _cache["nc"]


def kernel(**inputs) -> np.ndarray:
    from concourse.bass_utils import run_bass_kernel_spmd
    nc = _get_program()
    in_maps = shard_inputs(inputs, _B // NPAIR, _S, _E, _HD, _T)
    res = run_bass_kernel_spmd(nc, in_maps, list(range(NCORES)))
    out = np.asarray(res.results[0]["loss"], np.float32).reshape(())
    return out



# revision 5
# speedup vs baseline: 1.2714x; 1.2714x over previous
"""BiLSTM-CRF loss kernel for Trainium2 (8 NeuronCores, Bass/Tile).

Strategy (v2)
-------------
Cores 0-3 run the FORWARD LSTM direction, cores 4-7 the BACKWARD direction
(fed time-reversed x), each over 16 of the 64 sequences (data-parallel over
batch within each direction).  Pair (c, c+4) handles the same 16 sequences.

Per core, one fused chunked loop (CH timesteps per chunk):
  - GX chunk n+1 (x @ W_ih^T + bias) is computed into SBUF (bf16),
    interleaved with the recurrence steps of chunk n so the big matmuls
    fill tensor-engine bubbles left by the serial LSTM chain.
  - LSTM cell per step: gate order f,i,g,o; W_hh matmuls accumulate into
    PSUM, GX is added by identity-stationary matmuls per gate block so the
    activations read PSUM directly and start early (sigmoid(f,i) under the
    g/o matmuls).  h is written bf16 straight into an SBUF history buffer.
  - em partials per chunk from the SBUF h history; two masked slots are
    written to DRAM and pair-AllReduced (fwd slot / time-reversed bwd slot).
  - CRF: gold score via one-hot matmul reductions; partition function via
    probability-domain scan with a constant e^-kappa prescale folded into
    exp(em), two interleaved batch groups to hide semaphore latency, and a
    proper rescale only every R steps.  Final loss AllReduce over 8 cores.
"""

import sys

sys.path.insert(0, "/opt/trn_rl_repo")

import numpy as np
import ml_dtypes
from contextlib import ExitStack

import concourse.bass as bass
import concourse.bacc as bacc
import concourse.tile as tile
import concourse.mybir as mybir

F32 = mybir.dt.float32
BF16 = mybir.dt.bfloat16
I32 = mybir.dt.int32
AFT = mybir.ActivationFunctionType
ALU = mybir.AluOpType
AXL = mybir.AxisListType

NCORES = 8
NPAIR = 4  # fwd cores 0..3, bwd cores 4..7
KAPPA = 2.2  # CRF scan prescale: eem = exp(em - KAPPA)


# ---------------------------------------------------------------------------
# program builder (SPMD: one program, per-core divergence is data only)
# ---------------------------------------------------------------------------

def build_program(b, S, E, HD, T, B_full, CH=32, R=64, stop_after=None):
    """b: sequences per core; returns the Bass program."""
    KE = E // 128          # input-proj K tiles
    NH = HD // 128         # hidden K tiles (= h tiles)
    NM = 4 * NH            # gate m-tiles (permuted order f,i,g,o)
    SB = S * b             # (t, b) flattened column count
    W = NH * b             # per-step h column width  (64)
    SBc = CH * b           # columns per chunk         (512)
    NCHK = S // CH
    assert S % CH == 0 and CH % 2 == 0 and NM == 16 and CH >= 2 * NM // 2

    nc = bacc.Bacc("TRN2", target_bir_lowering=False, debug=False,
                   num_devices=NCORES)

    # ---- I/O ----
    xT = nc.dram_tensor("xT", [KE, 128, SB], BF16, kind="ExternalInput")
    wihT = nc.dram_tensor("wihT", [KE, 128, 4 * HD], BF16, kind="ExternalInput")
    whhT = nc.dram_tensor("whhT", [NH, 128, 4 * HD], BF16, kind="ExternalInput")
    bias4 = nc.dram_tensor("bias4", [128, NM], F32, kind="ExternalInput")
    ident = nc.dram_tensor("ident", [128, 128], BF16, kind="ExternalInput")
    wtagT = nc.dram_tensor("wtagT", [NH, 128, T], BF16, kind="ExternalInput")
    tagb = nc.dram_tensor("tagb", [T, 1], F32, kind="ExternalInput")
    m0 = nc.dram_tensor("m0", [T, 1], F32, kind="ExternalInput")
    m1 = nc.dram_tensor("m1", [T, 1], F32, kind="ExternalInput")
    labT = nc.dram_tensor("labT", [S, b], I32, kind="ExternalInput")
    transm = nc.dram_tensor("transm", [T, T], F32, kind="ExternalInput")
    startv = nc.dram_tensor("startv", [T, 1], F32, kind="ExternalInput")
    endv = nc.dram_tensor("endv", [T, 1], F32, kind="ExternalInput")
    loss = nc.dram_tensor("loss", [1, 1], F32, kind="ExternalOutput")

    with tile.TileContext(nc) as tc, ExitStack() as top:
        dram = top.enter_context(tc.tile_pool(name="dram", bufs=1, space="DRAM"))
        emdb = dram.tile([2, T, SB], F32)
        emdbo = dram.tile([2, T, SB], F32)
        lossdb = dram.tile([1, 1], F32)
        lossout = dram.tile([1, 1], F32)

        persist = top.enter_context(tc.tile_pool(name="persist", bufs=1))
        hist = persist.tile([128, S * W], BF16)      # h history [t, k, b]
        c_sb = persist.tile([128, W], F32)

        # ============== fused phase A+B+C (chunked) ==============
        ab = ExitStack()
        wp = ab.enter_context(tc.tile_pool(name="weights", bufs=1))
        wih_sb = wp.tile([128, KE * 4 * HD], BF16)
        whh_sb = wp.tile([128, NH * 4 * HD], BF16)
        bias_sb = wp.tile([128, NM], F32)
        ident_sb = wp.tile([128, 128], BF16)
        wtag_sb = wp.tile([128, NH * T], BF16)
        tagb_sb = wp.tile([T, 1], F32)
        m0_sb = wp.tile([T, 1], F32)
        m1_sb = wp.tile([T, 1], F32)
        nc.sync.dma_start(wih_sb[:], wihT[:])
        nc.sync.dma_start(whh_sb[:], whhT[:])
        nc.sync.dma_start(bias_sb[:], bias4[:])
        nc.sync.dma_start(ident_sb[:], ident[:])
        nc.sync.dma_start(wtag_sb[:], wtagT[:])
        nc.sync.dma_start(tagb_sb[:], tagb[:])
        nc.sync.dma_start(m0_sb[:], m0[:])
        nc.sync.dma_start(m1_sb[:], m1[:])

        xp = ab.enter_context(tc.tile_pool(name="xin", bufs=2))
        gxsp = ab.enter_context(tc.tile_pool(name="gxs", bufs=2))
        gxps = ab.enter_context(tc.tile_pool(name="gxps", bufs=2, space="PSUM"))
        rp = ab.enter_context(tc.tile_pool(name="recps", bufs=2, space="PSUM"))
        ep = ab.enter_context(tc.tile_pool(name="emps", bufs=2, space="PSUM"))
        tp = ab.enter_context(tc.tile_pool(name="steptmp", bufs=2))
        sp2 = ab.enter_context(tc.tile_pool(name="emtmp", bufs=2))

        def emit_gx_mtile(m, xt_sb, gxc):
            ps = gxps.tile([128, SBc], F32)
            for k in range(KE):
                nc.tensor.matmul(
                    ps[:],
                    wih_sb[:, k * 4 * HD + m * 128:k * 4 * HD + (m + 1) * 128],
                    xt_sb[:, k * SBc:(k + 1) * SBc],
                    start=(k == 0), stop=(k == KE - 1))
            nc.vector.tensor_scalar(gxc[:, m * SBc:(m + 1) * SBc], ps[:],
                                    bias_sb[:, m:m + 1], None, op0=ALU.add)

        # prologue: x + GX for chunk 0
        xt_sb = xp.tile([128, KE * SBc], BF16)
        nc.sync.dma_start(xt_sb[:], xT[:, :, 0:SBc])
        gxc = gxsp.tile([128, NM * SBc], BF16)
        for m in range(NM):
            emit_gx_mtile(m, xt_sb, gxc)

        for n in range(NCHK):
            gx_cur = gxc
            if n + 1 < NCHK:
                xt_sb = xp.tile([128, KE * SBc], BF16)
                nc.sync.dma_start(
                    xt_sb[:], xT[:, :, (n + 1) * SBc:(n + 2) * SBc])
                gxc = gxsp.tile([128, NM * SBc], BF16)

            gxv = gx_cur[:].rearrange("p (m c) -> p m c", m=NM)
            for tt in range(CH):
                t = n * CH + tt

                def gx_ap(mlo, mn):
                    return gxv[:, mlo:mlo + mn, tt * b:(tt + 1) * b]

                if t == 0:
                    sig = tp.tile([128, 3 * W], F32, tag="sig")
                    nc.scalar.activation(
                        sig[:, 0:2 * W].rearrange("p (m c) -> p m c", m=2 * NH),
                        gx_ap(0, 2 * NH), AFT.Sigmoid)
                    tg = tp.tile([128, W], F32, tag="tg")
                    nc.scalar.activation(
                        tg[:].rearrange("p (m c) -> p m c", m=NH),
                        gx_ap(2 * NH, NH), AFT.Tanh)
                    nc.scalar.activation(
                        sig[:, 2 * W:3 * W].rearrange("p (m c) -> p m c", m=NH),
                        gx_ap(3 * NH, NH), AFT.Sigmoid)
                    nc.vector.tensor_mul(c_sb[:], sig[:, W:2 * W], tg[:])
                else:
                    h_prev = hist[:, (t - 1) * W:t * W]
                    ps = rp.tile([128, NM * b], F32)

                    def cell_block(mlo, mn):
                        for mm in range(mlo, mlo + mn):
                            for kt in range(NH):
                                nc.tensor.matmul(
                                    ps[:, mm * b:(mm + 1) * b],
                                    whh_sb[:, kt * 4 * HD + mm * 128:
                                           kt * 4 * HD + (mm + 1) * 128],
                                    h_prev[:, kt * b:(kt + 1) * b],
                                    start=(kt == 0), stop=(kt == NH - 1))
                        nc.tensor.matmul(
                            ps[:, mlo * b:(mlo + mn) * b],
                            ident_sb[:], gx_ap(mlo, mn),
                            start=False, stop=True, skip_group_check=True)

                    cell_block(0, 2 * NH)         # f, i
                    sig = tp.tile([128, 3 * W], F32, tag="sig")
                    nc.scalar.activation(sig[:, 0:2 * W], ps[:, 0:2 * W],
                                         AFT.Sigmoid)
                    cell_block(2 * NH, NH)        # g
                    tg = tp.tile([128, W], F32, tag="tg")
                    nc.scalar.activation(tg[:], ps[:, 2 * W:3 * W], AFT.Tanh)
                    cell_block(3 * NH, NH)        # o
                    t1 = tp.tile([128, W], F32, tag="t1")
                    nc.vector.tensor_mul(t1[:], sig[:, 0:W], c_sb[:])
                    t2 = tp.tile([128, W], F32, tag="t2")
                    nc.vector.tensor_mul(t2[:], sig[:, W:2 * W], tg[:])
                    nc.vector.tensor_add(c_sb[:], t1[:], t2[:])
                    nc.scalar.activation(sig[:, 2 * W:3 * W], ps[:, 3 * W:4 * W],
                                         AFT.Sigmoid)
                tanc = tp.tile([128, W], F32, tag="tanc")
                nc.scalar.activation(tanc[:], c_sb[:], AFT.Tanh)
                nc.vector.tensor_mul(hist[:, t * W:(t + 1) * W],
                                     sig[:, 2 * W:3 * W], tanc[:])

                # interleave GX production for chunk n+1 into this chunk
                if n + 1 < NCHK and tt % 2 == 1 and tt // 2 < NM:
                    emit_gx_mtile(tt // 2, xt_sb, gxc)

            # ---- em partial for chunk n ----
            hv = hist[:, n * CH * W:(n + 1) * CH * W].rearrange(
                "p (t k c) -> p t k c", t=CH, k=NH)
            pse = ep.tile([T, SBc], F32)
            for kt in range(NH):
                nc.tensor.matmul(
                    pse[:].rearrange("p (t c) -> p t c", t=CH),
                    wtag_sb[:, kt * T:(kt + 1) * T],
                    hv[:, :, kt, :],
                    start=(kt == 0), stop=(kt == NH - 1))
            s0 = sp2.tile([T, SBc], F32, tag="s0")
            nc.vector.tensor_scalar(s0[:], pse[:], tagb_sb[:], m0_sb[:],
                                    op0=ALU.add, op1=ALU.mult)
            nc.sync.dma_start(emdb[0, :, n * SBc:(n + 1) * SBc], s0[:])
            s1 = sp2.tile([T, SBc], F32, tag="s1")
            nc.vector.tensor_scalar(s1[:], pse[:], tagb_sb[:], m1_sb[:],
                                    op0=ALU.add, op1=ALU.mult)
            nc.sync.dma_start(emdb[1, :, n * SBc:(n + 1) * SBc], s1[:])

        if stop_after == 'B':
            with tc.tile_pool(name="bail", bufs=1) as bp:
                bt = bp.tile([1, 1], F32)
                nc.vector.tensor_copy(bt[:], c_sb[0:1, 0:1])
                nc.sync.dma_start(loss[:], bt[:])
            ab.close()
            nc.compile()
            return nc

        ab.close()

        # ---- pair AllReduce of em slots ----
        nc.gpsimd.collective_compute(
            "AllReduce", ALU.add,
            replica_groups=[[c, c + NPAIR] for c in range(NPAIR)],
            ins=[emdb.opt()], outs=[emdbo.opt()])

        crf = top.enter_context(tc.tile_pool(name="crf", bufs=1))
        em_full = crf.tile([T, SB], F32, tag="emfull")
        eem = crf.tile([T, SB], F32, tag="eem")
        nc.sync.dma_start(em_full[:], emdbo[0])
        nc.sync.dma_start(
            eem[:], emdbo[1].rearrange("j (t c) -> j t c", t=S)[:, ::-1, :])
        nc.vector.tensor_add(em_full[:], em_full[:], eem[:])

        if stop_after == 'C':
            with tc.tile_pool(name="bail", bufs=1) as bp:
                bt = bp.tile([1, 1], F32)
                nc.vector.tensor_copy(bt[:], em_full[0:1, 0:1])
                nc.sync.dma_start(loss[:], bt[:])
            nc.compile()
            return nc

        # ---------------- Phase D: CRF ----------------
        with ExitStack() as ph:
            sp = ph.enter_context(tc.tile_pool(name="crftmp", bufs=2))
            big = ph.enter_context(tc.tile_pool(name="crfbig", bufs=2))
            ap_ = ph.enter_context(tc.tile_pool(name="alphas", bufs=2))

            cst = sp.tile([T, T], F32, tag="cst")        # transitions
            nc.sync.dma_start(cst[:], transm[:])
            st_sb = sp.tile([T, 1], F32, tag="stv")
            nc.sync.dma_start(st_sb[:], startv[:])
            en_sb = sp.tile([T, 1], F32, tag="env")
            nc.sync.dma_start(en_sb[:], endv[:])

            # --- numerator ---
            lab9 = big.tile([T, SB], I32, tag="big")
            nc.sync.dma_start(
                lab9[:],
                labT[:].rearrange("s c -> (s c)")[None, :].broadcast_to((T, SB)))
            labf = big.tile([T, SB], F32, tag="big")
            nc.vector.tensor_copy(labf[:], lab9[:])
            io9 = sp.tile([T, 1], I32, tag="io9")
            nc.gpsimd.iota(io9[:], pattern=[[0, 1]], base=0, channel_multiplier=1)
            io9f = sp.tile([T, 1], F32, tag="io9f")
            nc.vector.tensor_copy(io9f[:], io9[:])
            onehot = big.tile([T, SB], F32, tag="big")
            nc.vector.tensor_scalar(onehot[:], labf[:], io9f[:], None,
                                    op0=ALU.is_equal)
            gmul = big.tile([T, SB], F32, tag="big")
            nc.vector.tensor_mul(gmul[:], onehot[:], em_full[:])
            acc = sp.tile([T, b], F32, tag="acc")
            nc.vector.tensor_reduce(
                acc[:], gmul[:].rearrange("j (t c) -> j c t", c=b),
                op=ALU.add, axis=AXL.X)
            # start/end gold scores
            stsc = sp.tile([T, b], F32, tag="stsc")
            nc.vector.tensor_scalar_mul(stsc[:], onehot[:, 0:b], st_sb[:])
            nc.vector.tensor_add(acc[:], acc[:], stsc[:])
            ensc = sp.tile([T, b], F32, tag="ensc")
            nc.vector.tensor_scalar_mul(ensc[:], onehot[:, (S - 1) * b:S * b],
                                        en_sb[:])
            nc.vector.tensor_add(acc[:], acc[:], ensc[:])
            # transition gold scores: TH = T^T @ onehot ; V = TH * onehot_next
            numps = ExitStack()
            pp = numps.enter_context(
                tc.tile_pool(name="numps", bufs=2, space="PSUM"))
            for tc0 in range(0, S - 1, 32):
                tn = min(32, S - 1 - tc0)
                thp = pp.tile([T, 32 * b], F32, tag="thp")
                nc.tensor.matmul(thp[:, 0:tn * b], cst[:],
                                 onehot[:, tc0 * b:(tc0 + tn) * b],
                                 start=True, stop=True)
                v = sp.tile([T, 32 * b], F32, tag="v")
                nc.vector.tensor_mul(v[:, 0:tn * b], thp[:, 0:tn * b],
                                     onehot[:, (tc0 + 1) * b:(tc0 + 1 + tn) * b])
                vr = sp.tile([T, b], F32, tag="vr")
                nc.vector.tensor_reduce(
                    vr[:], v[:, 0:tn * b].rearrange("j (t c) -> j c t", c=b),
                    op=ALU.add, axis=AXL.X)
                nc.vector.tensor_add(acc[:], acc[:], vr[:])
            ones9 = sp.tile([T, 1], F32, tag="ones9")
            nc.vector.memset(ones9[:], 1.0)
            ones19 = sp.tile([1, T], F32, tag="ones19")
            nc.vector.memset(ones19[:], 1.0)
            nump = pp.tile([1, b], F32, tag="nump")
            nc.tensor.matmul(nump[:], ones9[:], acc[:], start=True, stop=True)
            num_sb = sp.tile([1, b], F32, tag="num")
            nc.vector.tensor_copy(num_sb[:], nump[:])
            numps.close()
            pp = ph.enter_context(tc.tile_pool(name="scanps", bufs=2, space="PSUM"))
            pp2 = ph.enter_context(tc.tile_pool(name="scanps2", bufs=2, space="PSUM"))

            # --- partition function (probability-domain scan, prescaled) ---
            NG = 2                     # interleaved batch groups
            gb = b // NG               # 8 sequences per group
            Em = sp.tile([T, T], F32, tag="Em")
            nc.scalar.activation(Em[:], cst[:], AFT.Exp)
            kneg = sp.tile([T, 1], F32, tag="kneg")
            nc.vector.memset(kneg[:], -KAPPA)
            nc.scalar.activation(eem[:], em_full[:], AFT.Exp, bias=kneg[:])
            es = sp.tile([T, 1], F32, tag="es")
            nc.scalar.activation(es[:], st_sb[:], AFT.Exp)
            ee = sp.tile([T, 1], F32, tag="ee")
            nc.scalar.activation(ee[:], en_sb[:], AFT.Exp)
            logacc = sp.tile([1, b], F32, tag="logacc")
            nc.vector.memset(logacc[:], 0.0)
            alphas = []
            for g in range(NG):
                al = ap_.tile([T, gb], F32, tag=f"al{g}")
                nc.vector.tensor_scalar_mul(al[:], eem[:, g * gb:(g + 1) * gb],
                                            es[:])
                alphas.append(al)
            for t in range(1, S):
                apss = []
                for g in range(NG):
                    aps = pp.tile([T, gb], F32, tag=f"aps{g}")
                    nc.tensor.matmul(aps[:], Em[:], alphas[g][:],
                                     start=True, stop=True)
                    apss.append(aps)
                for g in range(NG):
                    al = ap_.tile([T, gb], F32, tag=f"al{g}")
                    nc.vector.tensor_mul(
                        al[:], apss[g][:],
                        eem[:, t * b + g * gb:t * b + (g + 1) * gb])
                    alphas[g] = al
                if t % R == 0:
                    for g in range(NG):
                        ssum = pp2.tile([1, gb], F32, tag="ssum")
                        nc.tensor.matmul(ssum[:], ones9[:], alphas[g][:],
                                         start=True, stop=True)
                        ls = sp.tile([1, gb], F32, tag=f"ls{g}")
                        nc.scalar.activation(ls[:], ssum[:], AFT.Ln)
                        nc.vector.tensor_add(
                            logacc[:, g * gb:(g + 1) * gb],
                            logacc[:, g * gb:(g + 1) * gb], ls[:])
                        rc = sp.tile([1, gb], F32, tag=f"rc{g}")
                        nc.vector.reciprocal(rc[:], ssum[:])
                        bc = pp2.tile([T, gb], F32, tag="bc")
                        nc.tensor.matmul(bc[:], ones19[:], rc[:],
                                         start=True, stop=True)
                        al = ap_.tile([T, gb], F32, tag=f"al{g}")
                        nc.vector.tensor_mul(al[:], alphas[g][:], bc[:])
                        alphas[g] = al
            lv = sp.tile([1, b], F32, tag="lv")
            for g in range(NG):
                zp = pp.tile([1, gb], F32, tag=f"aps{g}")
                nc.tensor.matmul(zp[:], ee[:], alphas[g][:],
                                 start=True, stop=True)
                lz = sp.tile([1, gb], F32, tag=f"lz{g}")
                nc.scalar.activation(lz[:], zp[:], AFT.Ln)
                logz = sp.tile([1, gb], F32, tag=f"logz{g}")
                nc.vector.tensor_add(logz[:], lz[:],
                                     logacc[:, g * gb:(g + 1) * gb])
                # num - (logz + S*kappa)
                nc.vector.tensor_sub(lv[:, g * gb:(g + 1) * gb],
                                     num_sb[:, g * gb:(g + 1) * gb], logz[:])
            lvk = sp.tile([1, b], F32, tag="lvk")
            nc.vector.tensor_scalar_add(lvk[:], lv[:], -float(S) * KAPPA)
            tot = sp.tile([1, 1], F32, tag="tot")
            nc.vector.tensor_reduce(tot[:], lvk[:], op=ALU.add, axis=AXL.X)
            sc = sp.tile([1, 1], F32, tag="sc")
            nc.vector.tensor_scalar_mul(sc[:], tot[:], -1.0 / (2.0 * B_full))
            nc.sync.dma_start(lossdb[:], sc[:])
            nc.gpsimd.collective_compute(
                "AllReduce", ALU.add,
                replica_groups=[list(range(NCORES))],
                ins=[lossdb.opt()], outs=[lossout.opt()])
            lf = sp.tile([1, 1], F32, tag="lf")
            nc.sync.dma_start(lf[:], lossout[:])
            nc.sync.dma_start(loss[:], lf[:])

    nc.compile()
    return nc


# ---------------------------------------------------------------------------
# host-side sharding
# ---------------------------------------------------------------------------

def _perm_figo(HD):
    # torch gate order i,f,g,o -> f,i,g,o
    return np.concatenate([
        np.arange(HD, 2 * HD), np.arange(0, HD),
        np.arange(2 * HD, 3 * HD), np.arange(3 * HD, 4 * HD)])


def shard_inputs(inputs, b, S, E, HD, T):
    KE, NH = E // 128, HD // 128
    perm = _perm_figo(HD)
    bf = ml_dtypes.bfloat16
    x = np.asarray(inputs["x"], np.float32)
    labels = np.asarray(inputs["labels"]).astype(np.int32)
    trans = np.asarray(inputs["transitions"], np.float32)
    startv = np.asarray(inputs["start_trans"], np.float32).reshape(T, 1)
    endv = np.asarray(inputs["end_trans"], np.float32).reshape(T, 1)
    Wtag = np.asarray(inputs["W_tag"], np.float32)
    btag = np.asarray(inputs["b_tag"], np.float32).reshape(T, 1)
    identm = np.eye(128, dtype=np.float32).astype(bf)

    per_dir = {}
    for d, sfx in enumerate(("f", "b")):
        Wih = np.asarray(inputs[f"W_ih_{sfx}"], np.float32)[perm]
        Whh = np.asarray(inputs[f"W_hh_{sfx}"], np.float32)[perm]
        bias = (np.asarray(inputs[f"b_ih_{sfx}"], np.float32)
                + np.asarray(inputs[f"b_hh_{sfx}"], np.float32))[perm]
        per_dir[d] = dict(
            wihT=np.ascontiguousarray(
                Wih.T.reshape(KE, 128, 4 * HD)).astype(bf),
            whhT=np.ascontiguousarray(
                Whh.T.reshape(NH, 128, 4 * HD)).astype(bf),
            bias4=np.ascontiguousarray(
                bias.reshape(4 * NH, 128).T).astype(np.float32),
            wtagT=np.ascontiguousarray(
                Wtag[:, d * HD:(d + 1) * HD].T.reshape(NH, 128, T)).astype(bf),
            tagb=btag if d == 0 else np.zeros_like(btag),
            m0=np.full((T, 1), 1.0 - d, np.float32),
            m1=np.full((T, 1), float(d), np.float32),
        )

    in_maps = []
    for c in range(NCORES):
        d = c // NPAIR                      # 0 fwd, 1 bwd
        g = c % NPAIR                       # batch group
        xs = x[g * b:(g + 1) * b]           # (b, S, E)
        if d == 1:
            xs = xs[:, ::-1, :]
        xTc = np.ascontiguousarray(xs.transpose(2, 1, 0).reshape(KE, 128, S * b)
                                   ).astype(bf)
        m = dict(per_dir[d])
        m["xT"] = xTc
        m["labT"] = np.ascontiguousarray(labels[g * b:(g + 1) * b].T)
        m["transm"] = trans
        m["startv"] = startv
        m["endv"] = endv
        m["ident"] = identm
        in_maps.append(m)
    return in_maps


# ---------------------------------------------------------------------------
# entry point
# ---------------------------------------------------------------------------

_B, _S, _E, _HD, _T = 64, 512, 1024, 512, 9
_cache = {}


def _get_program():
    if "nc" not in _cache:
        _cache["nc"] = build_program(_B // NPAIR, _S, _E, _HD, _T, _B)
    return _cache["nc"]


def kernel(**inputs) -> np.ndarray:
    from concourse.bass_utils import run_bass_kernel_spmd
    nc = _get_program()
    in_maps = shard_inputs(inputs, _B // NPAIR, _S, _E, _HD, _T)
    res = run_bass_kernel_spmd(nc, in_maps, list(range(NCORES)))
    out = np.asarray(res.results[0]["loss"], np.float32).reshape(())
    return out
